# revision 12
# baseline (speedup 1.0000x reference)
"""Trainium2 Bass kernel for nn_CifarResNet (DT-conv ResNet, training-mode BN).

Sharding: data-parallel over batch across 8 NeuronCores (32 images/core);
params replicated; BN batch statistics via a small AllReduce per BN layer.

Per-core layout: activations live in SBUF as (128 partitions, ipg, Hp, Wp)
zero-padded buffers.  Partition (j, c) = image-group j, channel c:
  F1 (conv1+layer1): 8 groups x 16 ch, 4 imgs/group, 32x32 (padded 34x34)
  F2 (layer2):       4 groups x 32 ch, 8 imgs/group, 16x16 (padded 18x18)
  F3 (layer3):       2 groups x 64 ch, 16 imgs/group,  8x8 (padded 10x10)

A DT conv computes d2 = |p|^2 + |c|^2 - 2 p.c entirely in PSUM via
per-kernel-offset matmuls over shifted views of the padded buffer:
9 matmuls with block-diagonal (-2*centers) weights (fp32r) over A, plus
9 matmuls with block-diagonal ones weights (bf16) over A2 = A*A (the |p|^2
term).  |c|^2 is a per-partition bias added at PSUM eviction.  The per-pixel
standardization over channels uses two more matmuls with block ones weights
(mean over the group's M partitions, replicated back to all of them).
"""
import sys
sys.path.insert(0, '/opt/trn_rl_repo')
import numpy as np
import ml_dtypes

import concourse.bass as bass
import concourse.mybir as mybir
import concourse.tile as tile
from concourse.bass_utils import run_bass_kernel_spmd

f32 = mybir.dt.float32
f32r = mybir.dt.float32r
bf16 = mybir.dt.bfloat16
AOP = mybir.AluOpType
AFT = mybir.ActivationFunctionType
AXL = mybir.AxisListType

NCORES = 8
BLOC = 32
EPS = 1e-5
KOFF3 = [(ki, kj) for ki in range(3) for kj in range(3)]

F1 = dict(g=8, C=16, ipg=4, H=32)
F2 = dict(g=4, C=32, ipg=8, H=16)
F3 = dict(g=2, C=64, ipg=16, H=8)
for _F in (F1, F2, F3):
    _F['Hp'] = _F['H'] + 2
    _F['X'] = _F['ipg'] * _F['H'] ** 2
    _F['Xp'] = _F['ipg'] * _F['Hp'] ** 2
FKEY = {id(F1): 1, id(F2): 2, id(F3): 3}

USE_F32R = False  # fp32r for the big matmuls (flips to f32 if precision fails)
MMDT = f32r if USE_F32R else f32


# ---------------------------------------------------------------- host prep

def _block_diag(g, C, M, block):
    W = np.zeros((128, 128), np.float32)
    for j in range(g):
        W[j * C:(j + 1) * C, j * M:(j + 1) * M] = block
    return W


def _trans_w(g_in, C, g_out, M, block, parity):
    W = np.zeros((128, 128), np.float32)
    for jo in range(g_out):
        ji = 2 * jo + parity
        W[ji * C:(ji + 1) * C, jo * M:(jo + 1) * M] = block
    return W


def _centers_blocks(centers, C_in, C_pad, k):
    M = centers.shape[0]
    geom = KOFF3 if k == 3 else [(0, 0)]
    out = []
    for (ki, kj) in geom:
        b = np.zeros((C_pad, M), np.float32)
        for c in range(C_in):
            b[c] = centers[:, c * k * k + ki * k + kj]
        out.append(b)
    return out


def _col128(vec, F):
    g, M = F['g'], F['C']
    return np.tile(np.asarray(vec, np.float32).reshape(M), g)


class Plan:
    def __init__(self):
        self.wtiles = []      # list of (128,128) f32
        self.bftiles = []     # list of (128,128) f32 (converted to bf16 later)
        self.bfidx = {}
        self.vcols = []       # list of (128,) f32
        self.convs = []

    def addw(self, arr):
        self.wtiles.append(arr.astype(np.float32))
        return len(self.wtiles) - 1

    def addbf(self, name, arr):
        self.bfidx[name] = len(self.bftiles)
        self.bftiles.append(arr.astype(np.float32))

    def addv(self, col):
        self.vcols.append(np.asarray(col, np.float32))
        return len(self.vcols) - 1


def build_plan(params):
    P = Plan()

    # shared per-format tiles: pp (bf16), mu, var, gs (f32)
    for F, name in ((F1, 1), (F2, 2), (F3, 3)):
        g, C, M = F['g'], F['C'], F['C']
        P.addbf(f'pp{name}', _block_diag(g, C, M, np.ones((C, M), np.float32)))
    P.addbf('ppt12_0', _trans_w(8, 16, 4, 32, np.ones((16, 32), np.float32), 0))
    P.addbf('ppt12_1', _trans_w(8, 16, 4, 32, np.ones((16, 32), np.float32), 1))
    P.addbf('ppt23_0', _trans_w(4, 32, 2, 64, np.ones((32, 64), np.float32), 0))
    P.addbf('ppt23_1', _trans_w(4, 32, 2, 64, np.ones((32, 64), np.float32), 1))

    shared = {}
    for F, name in ((F1, 1), (F2, 2), (F3, 3)):
        g, M, C = F['g'], F['C'], F['C']
        shared[f'mu{name}'] = P.addw(
            _block_diag(g, M, M, np.full((M, M), 1.0 / M, np.float32)))
        shared[f'var{name}'] = P.addw(
            _block_diag(g, M, M, np.full((M, M), 1.0 / (M - 1), np.float32)))
        shared[f'gs{name}'] = P.addw(np.tile(np.eye(C, dtype=np.float32), (g, g)))
    fc_w = np.asarray(params['fc_w'], np.float32)
    fcw = np.zeros((128, 128), np.float32)
    for j in range(2):
        fcw[j * 64:(j + 1) * 64, j * 10:(j + 1) * 10] = fc_w.T / 64.0
    shared['fc'] = P.addw(fcw)
    fcb = np.zeros(128, np.float32)
    for j in range(2):
        fcb[j * 10:(j + 1) * 10] = np.asarray(params['fc_b'], np.float32)
    shared['fcb'] = P.addv(fcb)
    shared['eps'] = P.addv(np.full(128, EPS, np.float32))
    P.shared = shared

    def add_conv(name, Fi, Fo, dtp, bnp, stride, k, C_in_eff=None,
                 resid=None, relu=True, out='pad'):
        centers = np.asarray(dtp['centers'], np.float32)
        scaler = np.asarray(dtp['scaler'], np.float32).ravel()
        C_in = C_in_eff if C_in_eff is not None else Fi['C']
        blocks = _centers_blocks(centers, C_in, Fi['C'], k)
        trans = Fi['g'] != Fo['g']
        cfg = dict(name=name, Fi=Fi, Fo=Fo, stride=stride, k=k, trans=trans,
                   resid=resid, relu=relu, out=out)
        if trans:
            w0 = [P.addw(_trans_w(Fi['g'], Fi['C'], Fo['g'], Fo['C'],
                                  -2.0 * b, 0)) for b in blocks]
            w1 = [P.addw(_trans_w(Fi['g'], Fi['C'], Fo['g'], Fo['C'],
                                  -2.0 * b, 1)) for b in blocks]
            cfg['wt'] = (w0, w1)
            tk = '12' if Fi is F1 else '23'
            cfg['pp'] = (f'ppt{tk}_0', f'ppt{tk}_1')
        else:
            cfg['wt'] = ([P.addw(_block_diag(Fi['g'], Fi['C'], Fo['C'], -2.0 * b))
                          for b in blocks],)
            cfg['pp'] = (f'pp{FKEY[id(Fo)]}',)
        cc = (centers.astype(np.float64) ** 2).sum(1).astype(np.float32)
        cfg['cc'] = P.addv(_col128(cc, Fo))
        cfg['ns'] = P.addv(_col128(-scaler, Fo))
        cfg['n3s'] = P.addv(_col128(-3.0 * scaler, Fo))
        cfg['gam'] = P.addv(_col128(np.asarray(bnp['g'], np.float32), Fo))
        cfg['bet'] = P.addv(_col128(np.asarray(bnp['b'], np.float32), Fo))
        P.convs.append(cfg)
        return cfg

    pp = params
    add_conv('conv1', F1, F1, pp['conv1'], pp['bn1'], 1, 3, C_in_eff=3)
    for b in range(3):
        bp = pp['layer1'][b]
        add_conv(f'l1b{b}c1', F1, F1, bp['conv1'], bp['bn1'], 1, 3)
        add_conv(f'l1b{b}c2', F1, F1, bp['conv2'], bp['bn2'], 1, 3,
                 resid='pad')
    for lname, Fi0, Fo in (('layer2', F1, F2), ('layer3', F2, F3)):
        for b in range(3):
            bp = pp[lname][b]
            Fi = Fi0 if b == 0 else Fo
            if b == 0:
                add_conv(f'{lname}ds', Fi, Fo, bp['downsample']['conv'],
                         bp['downsample']['bn'], 2, 1, relu=False, out='flat')
                add_conv(f'{lname}b0c1', Fi, Fo, bp['conv1'], bp['bn1'], 2, 3)
                add_conv(f'{lname}b0c2', Fo, Fo, bp['conv2'], bp['bn2'], 1, 3,
                         resid='flat')
            else:
                add_conv(f'{lname}b{b}c1', Fo, Fo, bp['conv1'], bp['bn1'], 1, 3)
                add_conv(f'{lname}b{b}c2', Fo, Fo, bp['conv2'], bp['bn2'], 1, 3,
                         resid='pad')
    return P


# ------------------------------------------------------------- wait splitting

def split_multi_waits(nc, max_waits=1):
    n = 0
    for fn in nc.m.functions:
        for bb in fn.blocks:
            out, changed = [], False
            for inst in list(bb.instructions):
                si = inst.sync_info
                if si is not None and si.on_wait and len(si.on_wait) > max_waits:
                    for k, w in enumerate(list(si.on_wait)):
                        ev = mybir.InstEventSemaphore(
                            name=f"{inst.name}-sw{k}", ins=[], outs=[])
                        ev.engine = inst.engine
                        ev.sync_info = mybir.SyncInfo(on_wait=[w], on_update=[])
                        out.append(ev)
                        n += 1
                    inst.sync_info = mybir.SyncInfo(
                        on_wait=[], on_update=list(si.on_update or []))
                    changed = True
                out.append(inst)
            if changed:
                bb.instructions = out
    return n


# ------------------------------------------------------------------ builder

def _chunk_rhs(cfg, A, ch, ki, kj):
    """rhs view of padded buffer A for output chunk ch, kernel offset (ki,kj).
    For k=1 convs pass ki=kj=None."""
    Fi, Fo, k = cfg['Fi'], cfg['Fo'], cfg['k']
    if not cfg['trans']:
        s = 1
        if Fo is F1:
            i, r0 = ch // 2, (ch % 2) * 16
            return A[:, i:i + 1, ki + r0:ki + r0 + 16, kj:kj + 32]
        if Fo is F2:
            i0 = 2 * ch
            return A[:, i0:i0 + 2, ki:ki + 16, kj:kj + 16]
        i0 = 8 * ch
        return A[:, i0:i0 + 8, ki:ki + 8, kj:kj + 8]
    if Fi is F1:   # F1 -> F2 stride 2
        i0 = (ch % 2) * 2
        if k == 3:
            return A[:, i0:i0 + 2, ki:ki + 32:2, kj:kj + 32:2]
        return A[:, i0:i0 + 2, 1:33:2, 1:33:2]
    # F2 -> F3 stride 2
    if k == 3:
        return A[:, 0:8, ki:ki + 16:2, kj:kj + 16:2]
    return A[:, 0:8, 1:17:2, 1:17:2]


def _chunk_dest(Fo, A, ch):
    if Fo is F1:
        i, r0 = ch // 2, (ch % 2) * 16
        return A[:, i:i + 1, 1 + r0:1 + r0 + 16, 1:33]
    if Fo is F2:
        return A[:, 2 * ch:2 * ch + 2, 1:17, 1:17]
    return A[:, 8 * ch:8 * ch + 8, 1:9, 1:9]


def _e_dims(Fo):
    if Fo is F1:
        return dict(i=1, y=16, x=32)
    if Fo is F2:
        return dict(i=2, y=16, x=16)
    return dict(i=8, y=8, x=8)


class Emitter:
    def __init__(self, nc, tc, plan, exts):
        self.nc, self.tc, self.plan = nc, tc, plan
        self.exts = exts
        self.uid = 0

    def nm(self, s):
        self.uid += 1
        return f"{s}_{self.uid}"

    def vcol(self, i):
        return self.vec[:, i:i + 1]

    def setup_pools(self, ctx):
        tc = self.tc
        self.sb = ctx.enter_context(tc.tile_pool(name="sb", bufs=1))
        self.psum = ctx.enter_context(tc.tile_pool(name="psum", bufs=1, space="PSUM"))
        self.dram = ctx.enter_context(tc.tile_pool(name="dram", bufs=1, space="DRAM"))
        nc = self.nc
        # resident vec columns
        self.vec = self.sb.tile([128, len(self.plan.vcols)], f32,
                                name="vec", tag="vec", bufs=1)
        nc.sync.dma_start(self.vec[:], self.exts['vecs'][:])
        self.shared_tiles = {}

    def load_shared(self, keys_f32=(), keys_bf=()):
        nc = self.nc
        for key in keys_f32:
            idx = self.plan.shared[key]
            dt = MMDT if key[:2] in ('mu', 'va') else f32
            t = self.sb.tile([128, 128], dt, name=self.nm(f"sh_{key}"),
                             tag=f"sh_{key.rstrip('123')}", bufs=2)
            nc.sync.dma_start(t[:], self.exts['wall'][:, idx * 128:(idx + 1) * 128]
                              .bitcast(dt))
            self.shared_tiles[key] = t
        for key in keys_bf:
            idx = self.plan.bfidx[key]
            t = self.sb.tile([128, 128], bf16, name=self.nm(f"shb_{key}"),
                             tag="shb_" + ('ppt' if 'ppt' in key else 'pp'), bufs=2)
            nc.sync.dma_start(t[:], self.exts['wbf'][:, idx * 128:(idx + 1) * 128])
            self.shared_tiles[key] = t

    def new_padded(self, Fo, want_sq=True):
        nc = self.nc
        A = self.sb.tile([128, Fo['ipg'], Fo['Hp'], Fo['Hp']], MMDT,
                         name=self.nm("A"), tag="A", bufs=3)
        nc.gpsimd.memset(A[:].bitcast(f32), 0.0)
        A2 = None
        if want_sq:
            A2 = self.sb.tile([128, Fo['ipg'], Fo['Hp'], Fo['Hp']], bf16,
                              name=self.nm("A2"), tag="A2", bufs=2)
        return A, A2

    # ---------------- stage 1: conv + standardize + exp + local BN sums
    def stage1(self, cfg, A, A2):
        nc, plan = self.nc, self.plan
        Fo = cfg['Fo']
        X, nch = Fo['X'], Fo['X'] // 512
        geom = KOFF3 if cfg['k'] == 3 else [(None, None)]

        # conv weight tiles (per parity set, contiguous in wall)
        wts = []
        for pset in cfg['wt']:
            i0, n = pset[0], len(pset)
            assert pset == list(range(i0, i0 + n))
            t = self.sb.tile([128, n * 128], MMDT, name=self.nm(f"w_{cfg['name']}"),
                             tag="w", bufs=3)
            nc.sync.dma_start(t[:], self.exts['wall'][:, i0 * 128:(i0 + n) * 128]
                              .bitcast(MMDT))
            wts.append(t)

        mu_t = self.shared_tiles[f"mu{FKEY[id(Fo)]}"]
        var_t = self.shared_tiles[f"var{FKEY[id(Fo)]}"]
        gs_t = self.shared_tiles[f"gs{FKEY[id(Fo)]}"]
        cc = self.vcol(cfg['cc'])
        ns = self.vcol(cfg['ns'])
        n3s = self.vcol(cfg['n3s'])

        ebuf = self.sb.tile([128, X], f32, name=self.nm(f"e_{cfg['name']}"),
                            tag="e", bufs=2)
        Se = self.sb.tile([128, nch], f32, name=self.nm("Se"), tag="Se", bufs=2)
        Sq = self.sb.tile([128, nch], f32, name=self.nm("Sq"), tag="Sq", bufs=2)

        for ch in range(nch):
            if cfg['trans']:
                par = (ch // 2) if cfg['Fi'] is F1 else ch
            else:
                par = 0
            wt = wts[par]
            ppk = cfg['pp'][par] if cfg['trans'] else cfg['pp'][0]
            pp_t = self.shared_tiles[ppk]

            ps = self.psum.tile([128, 512], f32, name=self.nm("d2"), tag="d2", bufs=2)
            n_mm = 2 * len(geom)
            mi = 0
            for kidx, (ki, kj) in enumerate(geom):
                rhs_c = _chunk_rhs(cfg, A, ch, ki, kj)
                rhs_p = _chunk_rhs(cfg, A2, ch, ki, kj)
                nc.tensor.matmul(ps[:], wt[:, kidx * 128:(kidx + 1) * 128], rhs_c,
                                 start=(mi == 0), stop=(mi == n_mm - 1))
                mi += 1
                nc.tensor.matmul(ps[:], pp_t[:], rhs_p,
                                 start=(mi == 0), stop=(mi == n_mm - 1))
                mi += 1

            u = self.sb.tile([128, 512], f32, name=self.nm("u"), tag="u", bufs=3)
            nc.vector.tensor_scalar(u[:], ps[:], cc, 0.0, AOP.add, AOP.max)
            d = self.sb.tile([128, 512], MMDT, name=self.nm("d"), tag="d", bufs=2)
            nc.scalar.activation(d[:], u[:], AFT.Sqrt)
            mu_ps = self.psum.tile([128, 512], f32, name=self.nm("mu"), tag="mu", bufs=2)
            nc.tensor.matmul(mu_ps[:], mu_t[:], d[:], start=True, stop=True)
            y = self.sb.tile([128, 512], f32, name=self.nm("y"), tag="y", bufs=2)
            nc.vector.tensor_sub(y[:], d[:].bitcast(f32), mu_ps[:])
            sq = self.sb.tile([128, 512], MMDT, name=self.nm("sq"), tag="u", bufs=3)
            nc.vector.tensor_mul(sq[:], y[:], y[:])
            var_ps = self.psum.tile([128, 512], f32, name=self.nm("var"), tag="var", bufs=2)
            nc.tensor.matmul(var_ps[:], var_t[:], sq[:], start=True, stop=True)
            sg = self.sb.tile([128, 512], f32, name=self.nm("sg"), tag="sg", bufs=2)
            nc.scalar.activation(sg[:], var_ps[:], AFT.Sqrt)
            rs = self.sb.tile([128, 512], f32, name=self.nm("rs"), tag="rs", bufs=2)
            nc.vector.reciprocal(rs[:], sg[:])
            nc.vector.tensor_mul(y[:], y[:], rs[:])
            e_sl = ebuf[:, ch * 512:(ch + 1) * 512]
            nc.scalar.activation(e_sl, y[:], AFT.Exp, bias=n3s, scale=ns,
                                 accum_out=Se[:, ch:ch + 1])
            e2 = self.sb.tile([128, 512], f32, name=self.nm("e2"), tag="u", bufs=3)
            nc.vector.scalar_tensor_tensor(e2[:], e_sl, 1.0, e_sl, AOP.mult, AOP.mult,
                                           accum_out=Sq[:, ch:ch + 1])

        SQ = self.sb.tile([128, 2], f32, name=self.nm("SQ"), tag="SQ", bufs=2)
        nc.vector.tensor_reduce(SQ[:, 0:1], Se[:], AXL.X, AOP.add)
        nc.vector.tensor_reduce(SQ[:, 1:2], Sq[:], AXL.X, AOP.add)
        gs_ps = self.psum.tile([128, 2], f32, name=self.nm("gsps"), tag="gs", bufs=1)
        nc.tensor.matmul(gs_ps[:], gs_t[:], SQ[:], start=True, stop=True)
        gssb = self.sb.tile([128, 2], f32, name=self.nm("gssb"), tag="gssb", bufs=2)
        nc.vector.tensor_scalar(gssb[:], gs_ps[:], 1.0, None, AOP.mult)
        return dict(cfg=cfg, ebuf=ebuf, gssb=gssb, A=A, A2=A2)

    # ---------------- AllReduce a group of convs' BN sums
    def allreduce(self, handles):
        nc = self.nc
        n = len(handles)
        bi = self.dram.tile([128, 2 * n], f32, name=self.nm("arin"), tag="arin",
                            bufs=2)
        bo = self.dram.tile([128, 2 * n], f32, name=self.nm("arout"), tag="arout",
                            bufs=2, addr_space="Shared")
        for i, h in enumerate(handles):
            nc.sync.dma_start(bi[:, 2 * i:2 * i + 2], h['gssb'][:])
        nc.gpsimd.collective_compute(
            "AllReduce", AOP.add, replica_groups=[list(range(NCORES))],
            ins=[bi.opt()], outs=[bo.opt()])
        glob = self.sb.tile([128, 2 * n], f32, name=self.nm("glob"), tag="glob",
                            bufs=2)
        nc.sync.dma_start(glob[:], bo[:])
        for i, h in enumerate(handles):
            h['glob'] = glob[:, 2 * i:2 * i + 2]

    # ---------------- stage 2: BN affine (+resid) (+relu) -> output buffer
    def stage2(self, h, out_A=None, out_A2=None, out_flat=None, resid_buf=None):
        nc = self.nc
        cfg = h['cfg']
        Fo = cfg['Fo']
        X, nch = Fo['X'], Fo['X'] // 512
        N = float(NCORES * BLOC * Fo['H'] ** 2)
        g = h['glob']
        mean = self.sb.tile([128, 1], f32, name=self.nm("mean"), tag="b1", bufs=2)
        nc.vector.tensor_scalar(mean[:], g[:, 0:1], 1.0 / N, None, AOP.mult)
        q = self.sb.tile([128, 1], f32, name=self.nm("q"), tag="b2", bufs=2)
        nc.vector.tensor_scalar(q[:], g[:, 1:2], 1.0 / N, None, AOP.mult)
        m2 = self.sb.tile([128, 1], f32, name=self.nm("m2"), tag="b3", bufs=2)
        nc.vector.tensor_mul(m2[:], mean[:], mean[:])
        var = self.sb.tile([128, 1], f32, name=self.nm("bvar"), tag="b4", bufs=2)
        nc.vector.tensor_sub(var[:], q[:], m2[:])
        sg = self.sb.tile([128, 1], f32, name=self.nm("bsg"), tag="b5", bufs=2)
        nc.scalar.activation(sg[:], var[:], AFT.Sqrt,
                             bias=self.vcol(self.plan.shared['eps']))
        rstd = self.sb.tile([128, 1], f32, name=self.nm("rstd"), tag="b6", bufs=2)
        nc.vector.reciprocal(rstd[:], sg[:])
        scale = self.sb.tile([128, 1], f32, name=self.nm("scale"), tag="b7", bufs=2)
        nc.vector.tensor_mul(scale[:], self.vcol(cfg['gam']), rstd[:])
        t0 = self.sb.tile([128, 1], f32, name=self.nm("t0"), tag="b8", bufs=2)
        nc.vector.tensor_mul(t0[:], mean[:], scale[:])
        shift = self.sb.tile([128, 1], f32, name=self.nm("shift"), tag="b9", bufs=2)
        nc.vector.tensor_sub(shift[:], self.vcol(cfg['bet']), t0[:])

        ed = _e_dims(Fo)
        for ch in range(nch):
            e_sl = h['ebuf'][:, ch * 512:(ch + 1) * 512]
            e_r = e_sl.rearrange("p (i y x) -> p i y x", **ed)
            if cfg['out'] == 'flat':
                nc.scalar.activation(out_flat[:, ch * 512:(ch + 1) * 512], e_sl,
                                     AFT.Identity, bias=shift[:], scale=scale[:])
                continue
            dest = _chunk_dest(Fo, out_A, ch)
            if cfg['resid'] is None:
                nc.scalar.activation(dest, e_r, AFT.Relu,
                                     bias=shift[:], scale=scale[:])
            else:
                rt = self.sb.tile([128, 512], f32, name=self.nm("rt"), tag="u", bufs=3)
                if cfg['resid'] == 'pad':
                    # ScalarTensorTensor inputs are limited to <=3 dims:
                    # read the strided residual via per-image 3-D views.
                    if Fo is F1:
                        i, r0 = ch // 2, (ch % 2) * 16
                        views = [(0, 512,
                                  resid_buf[:, i, 1 + r0:1 + r0 + 16, 1:33])]
                    else:
                        H = Fo['H']
                        npx = H * H
                        n_img = 512 // npx
                        views = [(k * npx, npx,
                                  resid_buf[:, ch * n_img + k, 1:1 + H, 1:1 + H])
                                 for k in range(n_img)]
                    for off, npx, rv in views:
                        nc.vector.scalar_tensor_tensor(
                            rt[:, off:off + npx], e_sl[:, off:off + npx],
                            scale[:], rv.bitcast(f32), AOP.mult, AOP.add)
                else:
                    rv = resid_buf[:, ch * 512:(ch + 1) * 512]
                    nc.vector.scalar_tensor_tensor(
                        rt[:], e_sl, scale[:], rv, AOP.mult, AOP.add)
                nc.scalar.activation(dest, rt[:].rearrange("p (i y x) -> p i y x", **ed),
                                     AFT.Relu, bias=shift[:])
        if out_A2 is not None:
            nc.gpsimd.tensor_mul(out_A2[:], out_A[:].bitcast(f32), out_A[:].bitcast(f32))


def build_network(plan):
    nc = bass.Bass()
    exts = dict(
        x=nc.dram_tensor("x", [BLOC, 3, 32, 32], f32, kind="ExternalInput"),
        wall=nc.dram_tensor("wall", [128, len(plan.wtiles) * 128], f32,
                            kind="ExternalInput"),
        wbf=nc.dram_tensor("wbf", [128, len(plan.bftiles) * 128], bf16,
                           kind="ExternalInput"),
        vecs=nc.dram_tensor("vecs", [128, len(plan.vcols)], f32,
                            kind="ExternalInput"),
        out=nc.dram_tensor("out", [BLOC, 10], f32, kind="ExternalOutput"),
    )
    from contextlib import ExitStack
    with tile.TileContext(nc) as tc, ExitStack() as ctx:
        E = Emitter(nc, tc, plan, exts)
        E.setup_pools(ctx)
        E.load_shared(keys_f32=('mu1', 'var1', 'gs1'), keys_bf=('pp1',))

        # input image -> F1 padded buffer (dma aps limited to 3 dims:
        # one dma per (group, image))
        A, A2 = E.new_padded(F1)
        for j in range(8):
            for i in range(4):
                nc.sync.dma_start(A[16 * j:16 * j + 3, i, 1:33, 1:33],
                                  exts['x'][4 * j + i].bitcast(MMDT))
        nc.gpsimd.tensor_mul(A2[:], A[:].bitcast(f32), A[:].bitcast(f32))

        convs = {c['name']: c for c in plan.convs}

        def run_simple(cname, A, A2, resid_pad=None, resid_flat=None,
                       out='pad'):
            cfg = convs[cname]
            h = E.stage1(cfg, A, A2)
            E.allreduce([h])
            if out == 'pad':
                An, A2n = E.new_padded(cfg['Fo'])
                E.stage2(h, out_A=An, out_A2=A2n,
                         resid_buf=resid_pad if resid_pad is not None else resid_flat)
                return An, A2n
            else:
                flat = E.sb.tile([128, cfg['Fo']['X']], f32, name=E.nm("ids"),
                                 tag="ids", bufs=1)
                E.stage2(h, out_flat=flat)
                return flat

        A, A2 = run_simple('conv1', A, A2)
        for b in range(3):
            Ain, A2in = A, A2
            Am, A2m = run_simple(f'l1b{b}c1', Ain, A2in)
            A, A2 = run_simple(f'l1b{b}c2', Am, A2m, resid_pad=Ain)

        for lname, fshared in (('layer2', ('mu2', 'var2', 'gs2')),
                               ('layer3', ('mu3', 'var3', 'gs3'))):
            tk = '12' if lname == 'layer2' else '23'
            E.load_shared(keys_f32=fshared,
                          keys_bf=(f'pp{fshared[0][-1]}', f'ppt{tk}_0', f'ppt{tk}_1'))
            # transition block: ds + c1 share one AllReduce
            Ain, A2in = A, A2
            hds = E.stage1(convs[f'{lname}ds'], Ain, A2in)
            h1 = E.stage1(convs[f'{lname}b0c1'], Ain, A2in)
            E.allreduce([hds, h1])
            ids = E.sb.tile([128, convs[f'{lname}ds']['Fo']['X']], f32,
                            name=E.nm("ids"), tag="ids", bufs=1)
            E.stage2(hds, out_flat=ids)
            Fo = convs[f'{lname}b0c1']['Fo']
            Am, A2m = E.new_padded(Fo)
            E.stage2(h1, out_A=Am, out_A2=A2m)
            A, A2 = run_simple(f'{lname}b0c2', Am, A2m, resid_flat=ids)
            for b in (1, 2):
                Ain, A2in = A, A2
                Am, A2m = run_simple(f'{lname}b{b}c1', Ain, A2in)
                A, A2 = run_simple(f'{lname}b{b}c2', Am, A2m, resid_pad=Ain)

        # avgpool + fc
        E.load_shared(keys_f32=('fc',))
        pool = E.sb.tile([128, 16], f32, name="pool", tag="pool", bufs=1)
        nc.vector.tensor_reduce(pool[:], A[:, 0:16, 1:9, 1:9].bitcast(f32),
                                AXL.XY, AOP.add)
        fc_ps = E.psum.tile([128, 16], f32, name="fcps", tag="gs", bufs=1)
        fct = E.shared_tiles['fc']
        nc.tensor.matmul(fc_ps[0:20, :], fct[:, 0:20], pool[:], start=True, stop=True)
        lg = E.sb.tile([128, 16], f32, name="lg", tag="lg", bufs=1)
        nc.vector.tensor_scalar(lg[0:20, :], fc_ps[0:20, :],
                                E.vcol(plan.shared['fcb'])[0:20], None, AOP.add)
        for j in range(2):
            nc.sync.dma_start(
                exts['out'][16 * j:16 * (j + 1), :].rearrange("i o -> o i"),
                lg[10 * j:10 * (j + 1), 0:16])
    return nc


# ------------------------------------------------------------------- driver

_CACHE = {}


def _to_np(o):
    if isinstance(o, dict):
        return {k: _to_np(v) for k, v in o.items()}
    if isinstance(o, list):
        return [_to_np(v) for v in o]
    return np.asarray(o)


def kernel(x, params):
    x = np.asarray(x, np.float32)
    params = _to_np(params)
    if 'nc' not in _CACHE:
        plan = build_plan(params)
        nc = build_network(plan)
        split_multi_waits(nc)
        _CACHE['plan'] = plan
        _CACHE['nc'] = nc
    plan, nc = _CACHE['plan'], _CACHE['nc']
    # (re)build value blobs from this call's params
    plan2 = build_plan(params)
    wall = np.concatenate(plan2.wtiles, axis=1)
    wbf = np.concatenate(plan2.bftiles, axis=1).astype(ml_dtypes.bfloat16)
    vecs = np.stack(plan2.vcols, axis=1).astype(np.float32)
    in_maps = [dict(x=np.ascontiguousarray(x[i * BLOC:(i + 1) * BLOC]),
                    wall=wall, wbf=wbf, vecs=vecs) for i in range(NCORES)]
    res = run_bass_kernel_spmd(nc, in_maps, list(range(NCORES)))
    return np.concatenate([res.results[i]['out'] for i in range(NCORES)], axis=0)
